# revision 14
# baseline (speedup 1.0000x reference)
"""CoAttention kernel for Trainium2 (8 NeuronCores, data-parallel over batch).

Math (per sample): ta = relu(seq_a @ W + b), tb likewise.  The reference
mean-pools the [N, rv_len, M] affinity before softmax, and mean-pooling
commutes with the dot product:

    atob_scores[n, l] = mean_m( ta[n,l,:] . tb_all_tokens[m,:] )
                      = ta[n,l,:] . mean_m( tb_all_tokens[m,:] )

so each side only needs a dot with the *other side's per-sample mean
feature vector* — the 52M-element affinity tensor is never materialized.

v3 schedule.  Trace findings driving it: the DMA path is descriptor-rate
bound (~250-650ns per partition-row record per SDMA engine), so
per-partition rows must be fat and descriptor counts minimal; and the PE
queue is in-order, so a tail matmul whose input DMA is stuck behind bulk
traffic stalls all later FC matmuls.

- seq ships as fp16 (end-to-end rel-err ~4e-3 vs 2e-2 tolerance).
- sq01 (c0|c1) packs TWO samples per tile -> [128, 4*TPS] with 10KB
  rows; side b on sync, side a on gpsimd; 256 records per queue.
- c2 remainders ship per side as one [45, BPC*TPS] fat-row DMA on the
  scalar queue, where row 44 is all-ones: the FC bias is folded into
  the contraction (ta = [x|1] @ [W;b]), killing the [128,1]
  128-descriptor bias DMA.
- W ships as one [128, 3*DH] packed tile (128 records instead of 300);
  no identity matrix: the final transpose is a DVE 32x32 block
  transpose + block-permuted output DMAs.
- mask ships as an ADDITIVE f32 mask (0 / -1e9), so masking is one DVE
  add instead of memset+copy_predicated.
- scalar queue carries only: wpack, maskadd, sq2 x2, then per-sample
  tail smalls (scores reshape, out_w, weight rows) — a tail DMA never
  waits behind bulk.
- ~12 junk matmuls at t=0 warm the PE HAM clock gate (else the first
  ~3.4us of FC runs at 1.2GHz).
- weight broadcast is a PE ones-matmul into bank-sized PSUM chunks;
  DVE multiplies taT(fp16) into an fp16 tmp and does per-chunk
  segmented reduces into aoutT.
- PSUM: fc tag [128,1280] bufs=2 (6 banks, score matvec chunks ride
  the same ring), wbc tag [128,512] bufs=2 (2 banks) = exactly 8.
"""
import sys

sys.path.insert(0, "/opt/trn_rl_repo")

import numpy as np

import concourse.bacc as bacc
import concourse.tile as tile
from concourse import mybir

# Problem shape (hardcoded per contest contract)
BZ, RV, RL, DIN, DH = 32, 10, 128, 300, 128
NCORES = 8
BPC = BZ // NCORES            # samples per core: 4
TPS = RV * RL                 # tokens per sample: 1280
RPC = BPC * RV                # reviews per core: 40
NEG_INF = -1e9
KC2 = DIN - 2 * DH + 1        # c2 contraction rows incl. the ones row: 45

f32 = mybir.dt.float32
f32r = mybir.dt.float32r
f16 = mybir.dt.float16
AF = mybir.ActivationFunctionType
AX = mybir.AxisListType
ALU = mybir.AluOpType

# free-dim chunks of one sample's tokens (N <= 512 for one PSUM bank;
# chunk boundaries are review-aligned so segmented reduces stay clean)
NCH = [(0, 512), (512, 512), (1024, 256)]

_CACHE = {}


def _build(iters=1, serial=False, loop_n=0, stage=3):
    nc = bacc.Bacc("TRN2", target_bir_lowering=False, debug=False)

    sq01_d = {s: nc.dram_tensor(f"sq01_{s}", [2 * DH, 4 * TPS], f16,
                                kind="ExternalInput")
              for s in "ab"}
    sq2_d = {s: nc.dram_tensor(f"sq2_{s}", [KC2, BPC * TPS], f16,
                               kind="ExternalInput")
             for s in "ab"}
    mska_d = nc.dram_tensor("maskadd", [2 * RV, BPC * RL], f32,
                            kind="ExternalInput")
    w_d = nc.dram_tensor("wpack", [DH, 3 * DH], f16, kind="ExternalInput")

    out_v = {s: nc.dram_tensor(f"out_{s}", [RPC, DH], f32, kind="ExternalOutput")
             for s in "ab"}
    out_w = {s: nc.dram_tensor(f"outw_{s}", [RPC, RL], f32, kind="ExternalOutput")
             for s in "ab"}

    import contextlib
    outer_tc = tile.TileContext(nc) if not serial else None
    with (outer_tc if outer_tc is not None else contextlib.nullcontext()):
      for it_ in range(iters):
        pfx = f"i{it_}_" if iters > 1 else ""
        with (
            tile.TileContext(nc) if serial else contextlib.nullcontext()
        ) as maybe_tc:
          tc = maybe_tc if serial else outer_tc
          with (
            tc.For_i(0, loop_n, 1) if loop_n else contextlib.nullcontext()
          ):
           with (
            tc.tile_pool(name=pfx + "cst", bufs=1) as cst,
            tc.tile_pool(name=pfx + "seq", bufs=1) as seqp,
            tc.tile_pool(name=pfx + "ta", bufs=8) as tap,
            tc.tile_pool(name=pfx + "sm", bufs=1) as smp_pool,
            tc.tile_pool(name=pfx + "ps", bufs=2, space="PSUM") as ps,
        ):
            # ---- scalar (HWDGE) queue: lean consts, then the two c2
            # fat-row batches; per-sample tail smalls follow later.
            w_t = cst.tile([DH, 3 * DH], f16, tag="w", name=pfx + "wpack")
            nc.scalar.dma_start(w_t[:], w_d[:])
            mskf = cst.tile([2 * RV, BPC * RL], f32, tag="mska",
                            name=pfx + "mskf")
            nc.scalar.dma_start(mskf[:], mska_d[:])
            sq2 = {}
            for s in "ab":
                sq2[s] = seqp.tile([KC2, BPC * TPS], f16, tag="seq2",
                                   bufs=2, name=f"{pfx}sq2_{s}")
                nc.scalar.dma_start(sq2[s][:], sq2_d[s][:])

            def w_lhs(c):
                kw = DH if c < 2 else KC2
                return w_t[0:kw, c * DH:(c + 1) * DH]

            # ---- bulk seq stream: two-sample pair tiles, 10KB rows;
            # side b on sync, side a on gpsimd.
            sq01 = {}
            for p in range(BPC // 2):
                for s, q in (("b", nc.sync), ("a", nc.gpsimd)):
                    t01 = seqp.tile([DH, 4 * TPS], f16, tag="seq01",
                                    bufs=4, name=f"{pfx}sq01_{s}{p}")
                    q.dma_start(t01[:], sq01_d[s][p * DH:(p + 1) * DH, :])
                    sq01[(s, p)] = t01

            def sq_rhs(s, smp, c, n0, nw):
                if c < 2:
                    base = (smp % 2) * 2 * TPS + c * TPS
                    return sq01[(s, smp // 2)][:, base + n0:base + n0 + nw]
                return sq2[s][:, smp * TPS + n0:smp * TPS + n0 + nw]

            taT, acc, mean16, aoutT = {}, {}, {}, {}
            for s in "ab":
                acc[s] = cst.tile([DH, BPC], f32, tag=f"acc{s}", name=f"{pfx}acc_{s}")
                mean16[s] = cst.tile([DH, BPC], f16, tag=f"mean{s}",
                                     name=f"{pfx}mean_{s}")
                aoutT[s] = cst.tile([DH, RPC], f32, tag=f"aoutT{s}",
                                    name=f"{pfx}aoutT_{s}")
            w2d_full = cst.tile([2 * RV, BPC * RL], f32, tag="w2d",
                                name=pfx + "w2d_full")
            ones1 = cst.tile([1, DH], f32, tag="ones", name=pfx + "ones1")
            nc.vector.memset(ones1[:], 1.0)
            # identity for the epilogue PE transpose, built on-chip (no
            # 128-descriptor const DMA): ones everywhere, then keep only
            # the j == p diagonal via an affine iota select
            ones_sq = cst.tile([DH, DH], f32, tag="onsq", name=pfx + "ones_sq")
            nc.vector.memset(ones_sq[:], 1.0)
            ident_t = cst.tile([DH, DH], f32, tag="ident", name=pfx + "ident_t")
            nc.gpsimd.affine_select(
                ident_t[:], ones_sq[:], pattern=[[1, DH]],
                compare_op=ALU.is_equal, fill=0.0,
                base=0, channel_multiplier=-1)

            # ---- PE warm-up: ~5us of junk matmuls so HAM un-throttles
            # the clock gate before the real FC arrives (data lands ~6us
            # in).  Writes a pool slot the first FC start=True resets.
            wrhs = smp_pool.tile([DH, 512], f16, tag="wrhs", name=pfx + "wrhs")
            nc.vector.memset(wrhs[:], 0.0)
            if stage >= 1:
                wfc = ps.tile([DH, TPS], f32, tag="fc", bufs=2,
                              name=pfx + "warm_fc")
                for k in range(12):
                    nc.tensor.matmul(wfc[:, 0:512], w_lhs(0), wrhs[:],
                                     start=True, stop=True)

            other = {"a": "b", "b": "a"}

            def emit_fc_pair(smp):
                if stage < 1:
                    return
                pfc = {}
                for s in ("b", "a"):
                    pfc[s] = ps.tile([DH, TPS], f32, tag="fc", bufs=2,
                                     name=f"{pfx}pfc_{s}{smp}")
                    taT[(s, smp)] = tap.tile([DH, TPS], f16, tag="taT",
                                             name=f"{pfx}taT_{s}{smp}")
                # c-outer: 3 weight loads per sample pair instead of 18
                for c in range(3):
                    for s in ("b", "a"):
                        for n0, nw in NCH:
                            nc.tensor.matmul(
                                pfc[s][:, n0:n0 + nw],
                                w_lhs(c),
                                sq_rhs(s, smp, c, n0, nw),
                                start=(c == 0), stop=(c == 2))
                for s in ("b", "a"):
                    nc.scalar.activation(
                        taT[(s, smp)][:], pfc[s][:], AF.Relu,
                        accum_out=acc[s][:, smp:smp + 1])
                    nc.scalar.mul(mean16[s][:, smp:smp + 1],
                                  acc[s][:, smp:smp + 1], 1.0 / TPS)

            def emit_scores(smp):
                if stage < 2:
                    return
                # scores: M=1 fp16 matvec against the other side's mean,
                # in bank-sized PSUM chunks riding the fc tag ring -> one
                # [1, 2*TPS] row -> one DMA (scalar queue) into the
                # [2RV, RL] softmax layout
                srow = smp_pool.tile([1, 2 * TPS], f32, tag="srow", bufs=2,
                                     name=f"{pfx}srow_{smp}")
                for i, s in enumerate(("a", "b")):
                    for ci, (n0, nw) in enumerate(NCH):
                        pscc = ps.tile([1, 512], f32, tag="fc", bufs=2,
                                       name=f"{pfx}psc_{s}{smp}{ci}")
                        nc.tensor.matmul(
                            pscc[:, :nw],
                            mean16[other[s]][:, smp:smp + 1],
                            taT[(s, smp)][:, n0:n0 + nw])
                        nc.scalar.copy(srow[:, i * TPS + n0:i * TPS + n0 + nw],
                                       pscc[:, :nw])
                scs = smp_pool.tile([2 * RV, RL], f32, tag="scs", bufs=2,
                                    name=f"{pfx}scs_{smp}")
                nc.scalar.dma_start(scs[:], srow[:])
                return scs

            def emit_soft(smp, scs):
                # masked softmax: additive mask -> one DVE add, then
                # rowmax / exp+rowsum / normalize
                lgs = smp_pool.tile([2 * RV, RL], f32, tag="lgs", bufs=2,
                                    name=f"{pfx}lgs_{smp}")
                nc.vector.tensor_tensor(
                    out=lgs[:], in0=scs[:],
                    in1=mskf[:, smp * RL:(smp + 1) * RL], op=ALU.add)
                negmax = smp_pool.tile([2 * RV, 1], f32, tag="negmax", bufs=2,
                                       name=f"{pfx}negmax_{smp}")
                nc.vector.reduce_max(out=negmax[:], in_=lgs[:],
                                     axis=AX.X, negate=True)
                e2d = smp_pool.tile([2 * RV, RL], f32, tag="e2d", bufs=2,
                                    name=f"{pfx}e2d_{smp}")
                ssum = smp_pool.tile([2 * RV, 1], f32, tag="ssum", bufs=2,
                                     name=f"{pfx}ssum_{smp}")
                nc.scalar.activation(e2d[:], lgs[:], AF.Exp, bias=negmax[:],
                                     accum_out=ssum[:])
                rec = smp_pool.tile([2 * RV, 1], f32, tag="rec", bufs=2,
                                    name=f"{pfx}rec_{smp}")
                nc.vector.reciprocal(rec[:], ssum[:])
                nc.vector.tensor_scalar_mul(
                    w2d_full[:, smp * RL:(smp + 1) * RL], e2d[:], rec[:])
                # ship softmax weights + per-side flattened weight rows
                # (scalar queue; all small)
                wrow = {}
                for i, s in enumerate(("a", "b")):
                    nc.scalar.dma_start(
                        out_w[s][smp * RV:(smp + 1) * RV, :],
                        w2d_full[i * RV:(i + 1) * RV,
                                 smp * RL:(smp + 1) * RL])
                    wrow[s] = smp_pool.tile([1, TPS], f32, tag=f"wrow{s}",
                                            bufs=2, name=f"{pfx}wrow_{s}{smp}")
                    nc.scalar.dma_start(
                        wrow[s][:], w2d_full[i * RV:(i + 1) * RV,
                                             smp * RL:(smp + 1) * RL])
                return wrow

            def emit_wsum(smp, wrow):
                if stage < 3:
                    return
                # weighted sums: PE ones-matmul broadcasts the weight row
                # into bank-sized PSUM chunks; DVE multiplies with
                # taT(fp16) into fp16 tmp; per-chunk segmented reduces
                # (chunks are review-aligned: 4+4+2 reviews)
                for i, s in enumerate(("a", "b")):
                    tmp = smp_pool.tile([DH, TPS], f16, tag="tmp", bufs=2,
                                        name=f"{pfx}tmp_{s}{smp}")
                    for ci, (n0, nw) in enumerate(NCH):
                        wbc = ps.tile([DH, 512], f32, tag="wbc", bufs=2,
                                      name=f"{pfx}wbc_{s}{smp}{ci}")
                        nc.tensor.matmul(
                            wbc[:, :nw],
                            ones1[:].bitcast(f32r),
                            wrow[s][:, n0:n0 + nw].bitcast(f32r))
                        nc.vector.tensor_tensor(
                            out=tmp[:, n0:n0 + nw],
                            in0=taT[(s, smp)][:, n0:n0 + nw],
                            in1=wbc[:, :nw], op=ALU.mult)
                        nc.vector.reduce_sum(
                            out=aoutT[s][:, smp * RV + n0 // RL:
                                         smp * RV + (n0 + nw) // RL],
                            in_=tmp[:, n0:n0 + nw].rearrange(
                                "p (r l) -> p r l", l=RL),
                            axis=AX.X)

            # Emission order: FC(s)+scores(s) per sample; softmax(s-1)
            # right after scores(s-1)'s DMA is in flight; the PE
            # broadcast for sample s-1 slots after SC(s) so no PE
            # instruction waits on data newer than its own sample.
            scs_t, wrow_t = {}, {}
            for smp in range(BPC):
                emit_fc_pair(smp)
                scs_t[smp] = emit_scores(smp)
                if stage >= 2 and smp >= 1:
                    wrow_t[smp - 1] = emit_soft(smp - 1, scs_t[smp - 1])
                    emit_wsum(smp - 1, wrow_t[smp - 1])
            if stage >= 2:
                wrow_t[BPC - 1] = emit_soft(BPC - 1, scs_t[BPC - 1])
                emit_wsum(BPC - 1, wrow_t[BPC - 1])

            # ---- per-side epilogue: PE transpose (on-chip identity),
            # DVE copy out of PSUM, out_v on the idle sync queue
            for s in ("a", "b") if stage >= 3 else ():
                ptp = ps.tile([RPC, DH], f32, tag="wbc", bufs=2,
                              name=f"{pfx}ptp_{s}")
                nc.tensor.matmul(ptp[:], aoutT[s][:], ident_t[:],
                                 is_transpose=True)
                aout = smp_pool.tile([RPC, DH], f32, tag="aout",
                                     name=f"{pfx}aout_{s}")
                nc.vector.tensor_copy(aout[:], ptp[:])
                nc.sync.dma_start(out_v[s][:], aout[:])

    nc.compile()
    return nc


def build_in_maps(seq_a, seq_b, mask_a, mask_b, W, b):
    seq_a = np.asarray(seq_a, dtype=np.float32)
    seq_b = np.asarray(seq_b, dtype=np.float32)
    mask_a = np.asarray(mask_a, dtype=np.int32)
    mask_b = np.asarray(mask_b, dtype=np.int32)
    W = np.asarray(W, dtype=np.float32)
    b = np.asarray(b, dtype=np.float32)

    # W packed [128, 3*DH] fp16 with the bias folded in as c2 row 44
    wpack = np.zeros((DH, 3 * DH), dtype=np.float16)
    wpack[:, 0:DH] = W[0:DH]
    wpack[:, DH:2 * DH] = W[DH:2 * DH]
    wpack[0:DIN - 2 * DH, 2 * DH:3 * DH] = W[2 * DH:DIN]
    wpack[KC2 - 1, 2 * DH:3 * DH] = b

    in_maps = []
    for core in range(NCORES):
        b0 = core * BPC
        sl = {}
        for name, seq in (("a", seq_a), ("b", seq_b)):
            # [BPC, TPS, DIN] -> [BPC, DIN, TPS] fp16; c0|c1 of two
            # samples concatenated column-wise into [2*128, 4*TPS]; c2
            # batched across the 4 samples into [45, BPC*TPS] with a
            # trailing all-ones row (bias fold)
            chunk = (seq[b0:b0 + BPC].reshape(BPC, TPS, DIN)
                     .transpose(0, 2, 1).astype(np.float16))
            c01 = np.concatenate(
                [chunk[:, 0:DH, :], chunk[:, DH:2 * DH, :]], axis=2)
            sl[f"sq01_{name}"] = np.ascontiguousarray(np.concatenate(
                [np.concatenate([c01[2 * p], c01[2 * p + 1]], axis=1)
                 for p in range(2)], axis=0))
            c2 = chunk[:, 2 * DH:DIN, :].transpose(1, 0, 2).reshape(
                DIN - 2 * DH, BPC * TPS)
            sl[f"sq2_{name}"] = np.ascontiguousarray(np.concatenate(
                [c2, np.ones((1, BPC * TPS), dtype=np.float16)], axis=0))
        msk = np.concatenate([
            mask[b0:b0 + BPC].reshape(BPC, RV, RL).transpose(1, 0, 2)
            .reshape(RV, BPC * RL) for mask in (mask_a, mask_b)], axis=0)
        sl["maskadd"] = np.where(msk > 0, 0.0, NEG_INF).astype(np.float32)
        sl["wpack"] = wpack
        in_maps.append(sl)
    return in_maps


def kernel(seq_a, seq_b, mask_a, mask_b, W, b):
    if "nc" not in _CACHE:
        _CACHE["nc"] = _build()
    nc = _CACHE["nc"]
    in_maps = build_in_maps(seq_a, seq_b, mask_a, mask_b, W, b)

    from concourse.bass_utils import run_bass_kernel_spmd
    res = run_bass_kernel_spmd(nc, in_maps, core_ids=list(range(NCORES)))
    _CACHE["last_result"] = res

    a_out = np.concatenate([r["out_a"] for r in res.results], axis=0)
    b_out = np.concatenate([r["out_b"] for r in res.results], axis=0)
    atob_w = np.concatenate([r["outw_a"] for r in res.results], axis=0)
    btoa_w = np.concatenate([r["outw_b"] for r in res.results], axis=0)
    return (a_out, b_out, atob_w, btoa_w)


# revision 16
# speedup vs baseline: 1.0721x; 1.0721x over previous
"""CoAttention kernel for Trainium2 (8 NeuronCores, data-parallel over batch).

Math (per sample): ta = relu(seq_a @ W + b), tb likewise.  The reference
mean-pools the [N, rv_len, M] affinity before softmax, and mean-pooling
commutes with the dot product:

    atob_scores[n, l] = mean_m( ta[n,l,:] . tb_all_tokens[m,:] )
                      = ta[n,l,:] . mean_m( tb_all_tokens[m,:] )

so each side only needs a dot with the *other side's per-sample mean
feature vector* — the 52M-element affinity tensor is never materialized.

v3 schedule.  Trace findings driving it: the DMA path is descriptor-rate
bound (~250-650ns per partition-row record per SDMA engine), so
per-partition rows must be fat and descriptor counts minimal; and the PE
queue is in-order, so a tail matmul whose input DMA is stuck behind bulk
traffic stalls all later FC matmuls.

- seq ships as fp16 (end-to-end rel-err ~4e-3 vs 2e-2 tolerance).
- sq01 (c0|c1) packs TWO samples per tile -> [128, 4*TPS] with 10KB
  rows; side b on sync, side a on gpsimd; 256 records per queue.
- c2 remainders ship per side as one [45, BPC*TPS] fat-row DMA on the
  scalar queue, where row 44 is all-ones: the FC bias is folded into
  the contraction (ta = [x|1] @ [W;b]), killing the [128,1]
  128-descriptor bias DMA.
- W ships as one [128, 3*DH] packed tile (128 records instead of 300);
  no identity matrix: the final transpose is a DVE 32x32 block
  transpose + block-permuted output DMAs.
- mask ships as an ADDITIVE f32 mask (0 / -1e9), so masking is one DVE
  add instead of memset+copy_predicated.
- scalar queue carries only: wpack, maskadd, sq2 x2, then per-sample
  tail smalls (scores reshape, out_w, weight rows) — a tail DMA never
  waits behind bulk.
- ~12 junk matmuls at t=0 warm the PE HAM clock gate (else the first
  ~3.4us of FC runs at 1.2GHz).
- weight broadcast is a PE ones-matmul into bank-sized PSUM chunks;
  DVE multiplies taT(fp16) into an fp16 tmp and does per-chunk
  segmented reduces into aoutT.
- PSUM: fc tag [128,1280] bufs=2 (6 banks, score matvec chunks ride
  the same ring), wbc tag [128,512] bufs=2 (2 banks) = exactly 8.
"""
import sys

sys.path.insert(0, "/opt/trn_rl_repo")

import numpy as np

import concourse.bacc as bacc
import concourse.tile as tile
from concourse import mybir

# Problem shape (hardcoded per contest contract)
BZ, RV, RL, DIN, DH = 32, 10, 128, 300, 128
NCORES = 8
BPC = BZ // NCORES            # samples per core: 4
TPS = RV * RL                 # tokens per sample: 1280
RPC = BPC * RV                # reviews per core: 40
NEG_INF = -1e9
KC2 = DIN - 2 * DH + 1        # c2 contraction rows incl. the ones row: 45

f32 = mybir.dt.float32
f32r = mybir.dt.float32r
f16 = mybir.dt.float16
AF = mybir.ActivationFunctionType
AX = mybir.AxisListType
ALU = mybir.AluOpType

# free-dim chunks of one sample's tokens (N <= 512 for one PSUM bank;
# chunk boundaries are review-aligned so segmented reduces stay clean)
NCH = [(0, 512), (512, 512), (1024, 256)]

_CACHE = {}


def _build(iters=1, serial=False, loop_n=0, stage=3):
    nc = bacc.Bacc("TRN2", target_bir_lowering=False, debug=False)

    sq01_d = {s: nc.dram_tensor(f"sq01_{s}", [2 * DH, 4 * TPS], f16,
                                kind="ExternalInput")
              for s in "ab"}
    sq2_d = {s: nc.dram_tensor(f"sq2_{s}", [KC2, BPC * TPS], f16,
                               kind="ExternalInput")
             for s in "ab"}
    mska_d = nc.dram_tensor("maskadd", [2 * RV, BPC * RL], f32,
                            kind="ExternalInput")
    w_d = nc.dram_tensor("wpack", [DH, 3 * DH], f16, kind="ExternalInput")

    out_v = {s: nc.dram_tensor(f"out_{s}", [RPC, DH], f32, kind="ExternalOutput")
             for s in "ab"}
    out_w = {s: nc.dram_tensor(f"outw_{s}", [RPC, RL], f32, kind="ExternalOutput")
             for s in "ab"}

    import contextlib
    outer_tc = tile.TileContext(nc) if not serial else None
    with (outer_tc if outer_tc is not None else contextlib.nullcontext()):
      for it_ in range(iters):
        pfx = f"i{it_}_" if iters > 1 else ""
        with (
            tile.TileContext(nc) if serial else contextlib.nullcontext()
        ) as maybe_tc:
          tc = maybe_tc if serial else outer_tc
          with (
            tc.For_i(0, loop_n, 1) if loop_n else contextlib.nullcontext()
          ):
           with (
            tc.tile_pool(name=pfx + "cst", bufs=1) as cst,
            tc.tile_pool(name=pfx + "seq", bufs=1) as seqp,
            tc.tile_pool(name=pfx + "ta", bufs=8) as tap,
            tc.tile_pool(name=pfx + "sm", bufs=1) as smp_pool,
            tc.tile_pool(name=pfx + "ps", bufs=2, space="PSUM") as ps,
        ):
            # ---- scalar (HWDGE) queue: lean consts, then the two c2
            # fat-row batches; per-sample tail smalls follow later.
            w_t = cst.tile([DH, 3 * DH], f16, tag="w", name=pfx + "wpack")
            nc.scalar.dma_start(w_t[:], w_d[:])
            mskf = cst.tile([2 * RV, BPC * RL], f32, tag="mska",
                            name=pfx + "mskf")
            nc.scalar.dma_start(mskf[:], mska_d[:])

            def w_lhs(c):
                kw = DH if c < 2 else KC2
                return w_t[0:kw, c * DH:(c + 1) * DH]

            # ---- bulk seq stream: two-sample pair tiles, 10KB rows;
            # side b on sync, side a on gpsimd.  Each side's c2 batch
            # rides its own bulk queue BETWEEN pair0 and pair1 so FC(0)
            # unblocks early and the scalar queue stays small-only.
            sq01, sq2 = {}, {}
            qeng = {"b": nc.sync, "a": nc.gpsimd}
            for s in "ab":
                t01 = seqp.tile([DH, 4 * TPS], f16, tag="seq01",
                                bufs=4, name=f"{pfx}sq01_{s}0")
                qeng[s].dma_start(t01[:], sq01_d[s][0:DH, :])
                sq01[(s, 0)] = t01
            for s in "ab":
                sq2[s] = seqp.tile([KC2, BPC * TPS], f16, tag="seq2",
                                   bufs=2, name=f"{pfx}sq2_{s}")
                qeng[s].dma_start(sq2[s][:], sq2_d[s][:])
            for s in "ab":
                t01 = seqp.tile([DH, 4 * TPS], f16, tag="seq01",
                                bufs=4, name=f"{pfx}sq01_{s}1")
                qeng[s].dma_start(t01[:], sq01_d[s][DH:2 * DH, :])
                sq01[(s, 1)] = t01

            def sq_rhs(s, smp, c, n0, nw):
                if c < 2:
                    base = (smp % 2) * 2 * TPS + c * TPS
                    return sq01[(s, smp // 2)][:, base + n0:base + n0 + nw]
                return sq2[s][:, smp * TPS + n0:smp * TPS + n0 + nw]

            taT, acc, mean16, aoutT = {}, {}, {}, {}
            for s in "ab":
                acc[s] = cst.tile([DH, BPC], f32, tag=f"acc{s}", name=f"{pfx}acc_{s}")
                mean16[s] = cst.tile([DH, BPC], f16, tag=f"mean{s}",
                                     name=f"{pfx}mean_{s}")
                aoutT[s] = cst.tile([DH, RPC], f32, tag=f"aoutT{s}",
                                    name=f"{pfx}aoutT_{s}")
            w2d_full = cst.tile([2 * RV, BPC * RL], f32, tag="w2d",
                                name=pfx + "w2d_full")
            ones1 = cst.tile([1, DH], f32, tag="ones", name=pfx + "ones1")
            nc.vector.memset(ones1[:], 1.0)
            # identity for the epilogue PE transpose, built on-chip (no
            # 128-descriptor const DMA): ones everywhere, then keep only
            # the j == p diagonal via an affine iota select
            ones_sq = cst.tile([DH, DH], f32, tag="onsq", name=pfx + "ones_sq")
            nc.vector.memset(ones_sq[:], 1.0)
            ident_t = cst.tile([DH, DH], f32, tag="ident", name=pfx + "ident_t")
            nc.gpsimd.affine_select(
                ident_t[:], ones_sq[:], pattern=[[1, DH]],
                compare_op=ALU.is_equal, fill=0.0,
                base=0, channel_multiplier=-1)

            # ---- PE warm-up: ~5us of junk matmuls so HAM un-throttles
            # the clock gate before the real FC arrives (data lands ~8us
            # in).  No input DMA dependency (junk lhsT from a memset);
            # writes a pool slot the first FC start=True resets.
            wrhs = smp_pool.tile([DH, 512], f16, tag="wrhs", name=pfx + "wrhs")
            nc.vector.memset(wrhs[:], 0.0)
            if stage >= 1:
                wfc = ps.tile([DH, TPS], f32, tag="fc", bufs=2,
                              name=pfx + "warm_fc")
                for k in range(12):
                    nc.tensor.matmul(wfc[:, 0:512], wrhs[:, 0:DH], wrhs[:],
                                     start=True, stop=True)

            other = {"a": "b", "b": "a"}

            def emit_fc_pair(smp):
                if stage < 1:
                    return
                pfc = {}
                for s in ("b", "a"):
                    pfc[s] = ps.tile([DH, TPS], f32, tag="fc", bufs=2,
                                     name=f"{pfx}pfc_{s}{smp}")
                    taT[(s, smp)] = tap.tile([DH, TPS], f16, tag="taT",
                                             name=f"{pfx}taT_{s}{smp}")
                # c-outer: 3 weight loads per sample pair instead of 18
                for c in range(3):
                    for s in ("b", "a"):
                        for n0, nw in NCH:
                            nc.tensor.matmul(
                                pfc[s][:, n0:n0 + nw],
                                w_lhs(c),
                                sq_rhs(s, smp, c, n0, nw),
                                start=(c == 0), stop=(c == 2))
                for s in ("b", "a"):
                    nc.scalar.activation(
                        taT[(s, smp)][:], pfc[s][:], AF.Relu,
                        accum_out=acc[s][:, smp:smp + 1])
                    nc.scalar.mul(mean16[s][:, smp:smp + 1],
                                  acc[s][:, smp:smp + 1], 1.0 / TPS)

            def emit_scores(smp):
                if stage < 2:
                    return
                # scores: M=1 fp16 matvec against the other side's mean,
                # in bank-sized PSUM chunks riding the fc tag ring -> one
                # [1, 2*TPS] row -> one DMA (scalar queue) into the
                # [2RV, RL] softmax layout
                srow = smp_pool.tile([1, 2 * TPS], f32, tag="srow", bufs=2,
                                     name=f"{pfx}srow_{smp}")
                for i, s in enumerate(("a", "b")):
                    for ci, (n0, nw) in enumerate(NCH):
                        pscc = ps.tile([1, 512], f32, tag="fc", bufs=2,
                                       name=f"{pfx}psc_{s}{smp}{ci}")
                        nc.tensor.matmul(
                            pscc[:, :nw],
                            mean16[other[s]][:, smp:smp + 1],
                            taT[(s, smp)][:, n0:n0 + nw])
                        nc.scalar.copy(srow[:, i * TPS + n0:i * TPS + n0 + nw],
                                       pscc[:, :nw])
                scs = smp_pool.tile([2 * RV, RL], f32, tag="scs", bufs=2,
                                    name=f"{pfx}scs_{smp}")
                nc.scalar.dma_start(scs[:], srow[:])
                return scs

            def emit_soft(smp, scs):
                # masked softmax: additive mask -> one DVE add, then
                # rowmax / exp+rowsum / normalize
                lgs = smp_pool.tile([2 * RV, RL], f32, tag="lgs", bufs=2,
                                    name=f"{pfx}lgs_{smp}")
                nc.vector.tensor_tensor(
                    out=lgs[:], in0=scs[:],
                    in1=mskf[:, smp * RL:(smp + 1) * RL], op=ALU.add)
                negmax = smp_pool.tile([2 * RV, 1], f32, tag="negmax", bufs=2,
                                       name=f"{pfx}negmax_{smp}")
                nc.vector.reduce_max(out=negmax[:], in_=lgs[:],
                                     axis=AX.X, negate=True)
                e2d = smp_pool.tile([2 * RV, RL], f32, tag="e2d", bufs=2,
                                    name=f"{pfx}e2d_{smp}")
                ssum = smp_pool.tile([2 * RV, 1], f32, tag="ssum", bufs=2,
                                     name=f"{pfx}ssum_{smp}")
                nc.scalar.activation(e2d[:], lgs[:], AF.Exp, bias=negmax[:],
                                     accum_out=ssum[:])
                rec = smp_pool.tile([2 * RV, 1], f32, tag="rec", bufs=2,
                                    name=f"{pfx}rec_{smp}")
                nc.vector.reciprocal(rec[:], ssum[:])
                nc.vector.tensor_scalar_mul(
                    w2d_full[:, smp * RL:(smp + 1) * RL], e2d[:], rec[:])
                # ship softmax weights + per-side flattened weight rows
                # (scalar queue; all small)
                wrow = {}
                for i, s in enumerate(("a", "b")):
                    nc.scalar.dma_start(
                        out_w[s][smp * RV:(smp + 1) * RV, :],
                        w2d_full[i * RV:(i + 1) * RV,
                                 smp * RL:(smp + 1) * RL])
                    wrow[s] = smp_pool.tile([1, TPS], f32, tag=f"wrow{s}",
                                            bufs=2, name=f"{pfx}wrow_{s}{smp}")
                    nc.scalar.dma_start(
                        wrow[s][:], w2d_full[i * RV:(i + 1) * RV,
                                             smp * RL:(smp + 1) * RL])
                return wrow

            def emit_wsum(smp, wrow):
                if stage < 3:
                    return
                # weighted sums: PE ones-matmul broadcasts the weight row
                # into bank-sized PSUM chunks; DVE multiplies with
                # taT(fp16) into fp16 tmp; per-chunk segmented reduces
                # (chunks are review-aligned: 4+4+2 reviews)
                for i, s in enumerate(("a", "b")):
                    tmp = smp_pool.tile([DH, TPS], f16, tag="tmp", bufs=2,
                                        name=f"{pfx}tmp_{s}{smp}")
                    for ci, (n0, nw) in enumerate(NCH):
                        wbc = ps.tile([DH, 512], f32, tag="wbc", bufs=2,
                                      name=f"{pfx}wbc_{s}{smp}{ci}")
                        nc.tensor.matmul(
                            wbc[:, :nw],
                            ones1[:].bitcast(f32r),
                            wrow[s][:, n0:n0 + nw].bitcast(f32r))
                        nc.vector.tensor_tensor(
                            out=tmp[:, n0:n0 + nw],
                            in0=taT[(s, smp)][:, n0:n0 + nw],
                            in1=wbc[:, :nw], op=ALU.mult)
                        nc.vector.reduce_sum(
                            out=aoutT[s][:, smp * RV + n0 // RL:
                                         smp * RV + (n0 + nw) // RL],
                            in_=tmp[:, n0:n0 + nw].rearrange(
                                "p (r l) -> p r l", l=RL),
                            axis=AX.X)

            # Emission order: FC(s)+scores(s) per sample; softmax(s-1)
            # right after scores(s-1)'s DMA is in flight; the PE
            # broadcast for sample s-1 slots after SC(s) so no PE
            # instruction waits on data newer than its own sample.
            scs_t, wrow_t = {}, {}
            for smp in range(BPC):
                emit_fc_pair(smp)
                scs_t[smp] = emit_scores(smp)
                if stage >= 2 and smp >= 1:
                    wrow_t[smp - 1] = emit_soft(smp - 1, scs_t[smp - 1])
                    emit_wsum(smp - 1, wrow_t[smp - 1])
            if stage >= 2:
                wrow_t[BPC - 1] = emit_soft(BPC - 1, scs_t[BPC - 1])
                emit_wsum(BPC - 1, wrow_t[BPC - 1])

            # ---- per-side epilogue: PE transpose (on-chip identity),
            # DVE copy out of PSUM, out_v on the idle sync queue
            for s in ("a", "b") if stage >= 3 else ():
                ptp = ps.tile([RPC, DH], f32, tag="wbc", bufs=2,
                              name=f"{pfx}ptp_{s}")
                nc.tensor.matmul(ptp[:], aoutT[s][:], ident_t[:],
                                 is_transpose=True)
                aout = smp_pool.tile([RPC, DH], f32, tag="aout",
                                     name=f"{pfx}aout_{s}")
                nc.vector.tensor_copy(aout[:], ptp[:])
                nc.sync.dma_start(out_v[s][:], aout[:])

    nc.compile()
    return nc


def build_in_maps(seq_a, seq_b, mask_a, mask_b, W, b):
    seq_a = np.asarray(seq_a, dtype=np.float32)
    seq_b = np.asarray(seq_b, dtype=np.float32)
    mask_a = np.asarray(mask_a, dtype=np.int32)
    mask_b = np.asarray(mask_b, dtype=np.int32)
    W = np.asarray(W, dtype=np.float32)
    b = np.asarray(b, dtype=np.float32)

    # W packed [128, 3*DH] fp16 with the bias folded in as c2 row 44
    wpack = np.zeros((DH, 3 * DH), dtype=np.float16)
    wpack[:, 0:DH] = W[0:DH]
    wpack[:, DH:2 * DH] = W[DH:2 * DH]
    wpack[0:DIN - 2 * DH, 2 * DH:3 * DH] = W[2 * DH:DIN]
    wpack[KC2 - 1, 2 * DH:3 * DH] = b

    in_maps = []
    for core in range(NCORES):
        b0 = core * BPC
        sl = {}
        for name, seq in (("a", seq_a), ("b", seq_b)):
            # [BPC, TPS, DIN] -> [BPC, DIN, TPS] fp16; c0|c1 of two
            # samples concatenated column-wise into [2*128, 4*TPS]; c2
            # batched across the 4 samples into [45, BPC*TPS] with a
            # trailing all-ones row (bias fold)
            chunk = (seq[b0:b0 + BPC].reshape(BPC, TPS, DIN)
                     .transpose(0, 2, 1).astype(np.float16))
            c01 = np.concatenate(
                [chunk[:, 0:DH, :], chunk[:, DH:2 * DH, :]], axis=2)
            sl[f"sq01_{name}"] = np.ascontiguousarray(np.concatenate(
                [np.concatenate([c01[2 * p], c01[2 * p + 1]], axis=1)
                 for p in range(2)], axis=0))
            c2 = chunk[:, 2 * DH:DIN, :].transpose(1, 0, 2).reshape(
                DIN - 2 * DH, BPC * TPS)
            sl[f"sq2_{name}"] = np.ascontiguousarray(np.concatenate(
                [c2, np.ones((1, BPC * TPS), dtype=np.float16)], axis=0))
        msk = np.concatenate([
            mask[b0:b0 + BPC].reshape(BPC, RV, RL).transpose(1, 0, 2)
            .reshape(RV, BPC * RL) for mask in (mask_a, mask_b)], axis=0)
        sl["maskadd"] = np.where(msk > 0, 0.0, NEG_INF).astype(np.float32)
        sl["wpack"] = wpack
        in_maps.append(sl)
    return in_maps


def kernel(seq_a, seq_b, mask_a, mask_b, W, b):
    if "nc" not in _CACHE:
        _CACHE["nc"] = _build()
    nc = _CACHE["nc"]
    in_maps = build_in_maps(seq_a, seq_b, mask_a, mask_b, W, b)

    from concourse.bass_utils import run_bass_kernel_spmd
    res = run_bass_kernel_spmd(nc, in_maps, core_ids=list(range(NCORES)))
    _CACHE["last_result"] = res

    a_out = np.concatenate([r["out_a"] for r in res.results], axis=0)
    b_out = np.concatenate([r["out_b"] for r in res.results], axis=0)
    atob_w = np.concatenate([r["outw_a"] for r in res.results], axis=0)
    btoa_w = np.concatenate([r["outw_b"] for r in res.results], axis=0)
    return (a_out, b_out, atob_w, btoa_w)
